# revision 1
# baseline (speedup 1.0000x reference)
"""Trainium2 Bass kernel for a single-layer transformer block (attention + FFN + 2x LayerNorm).

Shapes (hardcoded): q,k,v [4,4096,128] fp32; w1 [128,512]; w2 [512,128]; out [4,4096,128].

Sharding: 8 cores; core c handles batch c//2, q-rows half c%2 (2048 rows each).
k/v for the batch are replicated on both cores of the pair. Pure data-parallel SPMD,
no collectives.

Per-core algorithm (activations kept TRANSPOSED: [feature/kpos on partitions, rows free]):
  - qT, kT built on-chip via PE transposes; v stays natural ([kpos,128] tiles = lhsT for P@v).
  - scores_T[kpos, rows] = kT_blk.T @ qT  (PE, contraction d=128)
  - P = exp(scores / sqrt(d))             (ACT; max-subtraction unneeded: logits ~N(0,1);
                                           softmax denominator cancels in LayerNorm
                                           scale-invariance)
  - attn_T[d, rows] += v_blk.T.T @ P_blk  (PE accumulation over 32 kpos blocks)
  - LN1 over d (=partitions): stats via ones-matmul (PE), rstd = exp(-0.5*ln(var+eps))
    (ACT, single table set), partition-broadcast via K=1 PE matmul, apply on DVE.
  - FFN: h1T = w1.T @ xT (+b1, relu on DVE), ffnT = w2_blk.T @ h1T accumulated (PE).
  - residual + LN2, PE-transpose back to natural layout, DMA out.

The emission is software-pipelined: the post-attention ops of block qb-1 are spread
between the attention matmul/exp stream of block qb so every engine queue stays busy
and the PE never idles long enough for the HAM clock gate to re-throttle.

Matmul operands are float32r (fast PE mode; walrus requires producers to round, so
those SBUF tiles are f32r-typed and DMA-loaded operands get a cast copy).
Env knobs for A/B testing: KERNEL_F32R=0 -> pure fp32; KERNEL_ATTN_DT=bf16 -> bf16
attention matmuls (faster weight load, lower precision).
"""

import os
import sys

sys.path.insert(0, "/opt/trn_rl_repo")

from collections import deque
from contextlib import ExitStack

import numpy as np

import concourse.bass as bass  # noqa: F401
from concourse import bacc
import concourse.tile as tile
import concourse.mybir as mybir
from concourse.bass_utils import run_bass_kernel_spmd
from concourse.masks import make_identity

B, S, D, F = 4, 4096, 128, 512
N_CORES = 8
HALF = S // 2          # q rows per core
QBLK = 512             # q rows per block (psum bank free width in fp32)
NQB = HALF // QBLK     # 4 q blocks per core
NKT = S // 128         # 32 kpos tiles
NQT = HALF // 128      # 16 q row tiles
FBLK = F // 128        # 4 FFN chunks
EPS = 1e-5
INV_SQRT_D = float(1.0 / np.sqrt(D))

f32 = mybir.dt.float32
f32r = mybir.dt.float32r
bf16 = mybir.dt.bfloat16
AF = mybir.ActivationFunctionType
ALU = mybir.AluOpType

USE_F32R = os.environ.get("KERNEL_F32R", "1") == "1"
MMDT = f32r if USE_F32R else f32
ATTN_DT = {"f32r": MMDT, "bf16": bf16, "f32": f32}[os.environ.get("KERNEL_ATTN_DT", "f32r")]


def _emit(nc, tc, ctx):
    q = nc.dram_tensor("q", [HALF, D], f32, kind="ExternalInput")
    k = nc.dram_tensor("k", [S, D], f32, kind="ExternalInput")
    v = nc.dram_tensor("v", [S, D], f32, kind="ExternalInput")
    w1 = nc.dram_tensor("w1", [D, F], f32, kind="ExternalInput")
    b1 = nc.dram_tensor("b1", [F], f32, kind="ExternalInput")
    w2 = nc.dram_tensor("w2", [F, D], f32, kind="ExternalInput")
    b2 = nc.dram_tensor("b2", [D], f32, kind="ExternalInput")
    g1 = nc.dram_tensor("g1", [D], f32, kind="ExternalInput")
    be1 = nc.dram_tensor("be1", [D], f32, kind="ExternalInput")
    g2 = nc.dram_tensor("g2", [D], f32, kind="ExternalInput")
    be2 = nc.dram_tensor("be2", [D], f32, kind="ExternalInput")
    out = nc.dram_tensor("out", [HALF, D], f32, kind="ExternalOutput")

    # ---------------- pools ----------------
    persist = ctx.enter_context(tc.tile_pool(name="persist", bufs=1))
    p_pool = ctx.enter_context(tc.tile_pool(name="p", bufs=6))
    xz_pool = ctx.enter_context(tc.tile_pool(name="xz", bufs=6))
    x_pool = ctx.enter_context(tc.tile_pool(name="x", bufs=4))
    h_pool = ctx.enter_context(tc.tile_pool(name="h", bufs=4))
    st_pool = ctx.enter_context(tc.tile_pool(name="st", bufs=3))
    y_pool = ctx.enter_context(tc.tile_pool(name="y", bufs=3))
    o_pool = ctx.enter_context(tc.tile_pool(name="o", bufs=4))

    score_ps = ctx.enter_context(tc.tile_pool(name="score_ps", bufs=2, space="PSUM"))
    acc_ps = ctx.enter_context(tc.tile_pool(name="acc_ps", bufs=2, space="PSUM"))
    misc_ps = ctx.enter_context(tc.tile_pool(name="misc_ps", bufs=2, space="PSUM"))

    # ---------------- constants ----------------
    ident = persist.tile([128, 128], f32, tag="ident")
    make_identity(nc, ident)
    ones_f32 = persist.tile([128, 1], f32, tag="ones_f32")
    nc.gpsimd.memset(ones_f32, 1.0 / D)
    ones_stat = persist.tile([128, 1], MMDT, tag="ones_stat")
    nc.vector.tensor_copy(ones_stat, ones_f32)
    onesb_f32 = persist.tile([1, 128], f32, tag="onesb_f32")
    nc.gpsimd.memset(onesb_f32, 1.0)
    ones_bc = persist.tile([1, 128], MMDT, tag="ones_bc")
    nc.vector.tensor_copy(ones_bc, onesb_f32)
    eps_t = persist.tile([1, 1], f32, tag="eps_t")
    nc.gpsimd.memset(eps_t, EPS)

    # First DMAs on the queue = the ones the first attention slot needs.
    KCH = 8                       # k/v tiles per DMA chunk
    v_r = v.rearrange("(t p) d -> p t d", p=128)
    k_r = k.rearrange("(t p) d -> p t d", p=128)
    q_r = q.rearrange("(t p) d -> p t d", p=128)
    v_f = persist.tile([128, NKT, 128], f32, tag="v_f")
    v_sb = persist.tile([128, NKT, 128], ATTN_DT, tag="v_sb")
    k_stage = persist.tile([128, NKT, 128], f32, tag="k_stage")
    q_stage = persist.tile([128, NQT, 128], f32, tag="q_stage")
    kT = persist.tile([128, S], ATTN_DT, tag="kT")
    qT = persist.tile([128, HALF], ATTN_DT, tag="qT")

    nc.sync.dma_start(out=q_stage[:, 0:8, :], in_=q_r[:, 0:8, :])
    nc.sync.dma_start(out=k_stage[:, 0:KCH, :], in_=k_r[:, 0:KCH, :])
    nc.sync.dma_start(out=v_f[:, 0:KCH, :], in_=v_r[:, 0:KCH, :])

    g1_t = persist.tile([128, 1], f32, tag="g1_t")
    nc.sync.dma_start(out=g1_t, in_=g1.ap().unsqueeze(1))
    be1_t = persist.tile([128, 1], f32, tag="be1_t")
    nc.sync.dma_start(out=be1_t, in_=be1.ap().unsqueeze(1))
    g2_t = persist.tile([128, 1], f32, tag="g2_t")
    nc.sync.dma_start(out=g2_t, in_=g2.ap().unsqueeze(1))
    be2_t = persist.tile([128, 1], f32, tag="be2_t")
    nc.sync.dma_start(out=be2_t, in_=be2.ap().unsqueeze(1))
    b2_t = persist.tile([128, 1], f32, tag="b2_t")
    nc.sync.dma_start(out=b2_t, in_=b2.ap().unsqueeze(1))

    w1_f = persist.tile([128, F], f32, tag="w1_f")
    nc.sync.dma_start(out=w1_f, in_=w1[:, :])
    w1_sb = persist.tile([128, F], MMDT, tag="w1_sb")
    nc.vector.tensor_copy(w1_sb, w1_f)

    w2_f = persist.tile([128, FBLK, D], f32, tag="w2_f")
    nc.sync.dma_start(out=w2_f, in_=w2.rearrange("(t p) d -> p t d", p=128))
    w2_sb = persist.tile([128, FBLK, D], MMDT, tag="w2_sb")
    nc.vector.tensor_copy(w2_sb, w2_f)

    b1_sb = persist.tile([128, FBLK], f32, tag="b1_sb")
    nc.sync.dma_start(out=b1_sb, in_=b1.rearrange("(t p) -> p t", p=128))

    # ---------------- remaining big-activation DMA chunks ----------------
    for c in range(1, NKT // KCH):
        s = slice(c * KCH, (c + 1) * KCH)
        nc.sync.dma_start(out=k_stage[:, s, :], in_=k_r[:, s, :])
        nc.sync.dma_start(out=v_f[:, s, :], in_=v_r[:, s, :])

    def transpose_tile(dst, stage_t, t):
        ps_t = misc_ps.tile([128, 128], f32, tag="misc", name="ps_t")
        nc.tensor.transpose(ps_t, stage_t[:, t, :], ident)
        nc.vector.tensor_copy(dst[:, t * 128 : (t + 1) * 128], ps_t)

    for t in range(4):  # block 0's q columns
        transpose_tile(qT, q_stage, t)

    # ---------------- post-attention phase as spreadable op list ----------------
    def layer_norm_T_ops(src_x, src_sq, g_t, be_t, dst):
        """Closures computing LN over the partition dim; src/dst are SBUF APs [128, n]."""
        ncols = src_x.shape[-1]
        state = {}

        def s1():
            state["mu"] = mu = misc_ps.tile([1, ncols], f32, tag="misc", name="ps_mu")
            nc.tensor.matmul(mu, ones_stat, src_x)

        def s2():
            state["ms"] = ms = misc_ps.tile([1, ncols], f32, tag="misc", name="ps_ms")
            nc.tensor.matmul(ms, ones_stat, src_sq)

        def s3():
            state["st"] = st = st_pool.tile([1, 2, ncols], MMDT, tag="st", name="st")
            nc.vector.tensor_copy(st[:, 0, :], state["mu"])
            nc.vector.tensor_tensor(st[:, 1, :], st[:, 0, :], st[:, 0, :], ALU.mult)
            nc.vector.tensor_tensor(st[:, 1, :], state["ms"], st[:, 1, :], ALU.subtract)

        def s4():
            st = state["st"]
            # rstd = exp(-0.5 * ln(var + eps)); Ln+Exp share one ACT table set.
            nc.scalar.activation(st[:, 1, :], st[:, 1, :], AF.Ln, bias=eps_t)
            nc.scalar.activation(st[:, 1, :], st[:, 1, :], AF.Exp, scale=-0.5)

        def s5():
            # broadcast mu/rstd across partitions via K=1 matmul (PE hop is far
            # cheaper than a GPSIMD partition_broadcast's semaphore latency)
            state["pbm"] = pbm = misc_ps.tile([128, ncols], f32, tag="misc", name="pbm")
            nc.tensor.matmul(pbm, ones_bc, state["st"][:, 0, :])
            state["pbr"] = pbr = misc_ps.tile([128, ncols], f32, tag="misc", name="pbr")
            nc.tensor.matmul(pbr, ones_bc, state["st"][:, 1, :])

        def s6():
            nc.vector.tensor_tensor(dst, src_x, state["pbm"], ALU.subtract)
            nc.vector.scalar_tensor_tensor(
                dst, dst, g_t, state["pbr"], ALU.mult, ALU.mult
            )
            nc.vector.tensor_scalar_add(dst, dst, be_t)

        return [s1, s2, s3, s4, s5, s6]

    def make_post_ops(qb, xz, x, c0, c1):
        """Closures for LN1 + FFN + residual + LN2 + store of columns [c0:c1) of
        block qb. xz ([128,2,QBLK]: x and x^2 in SBUF) is produced eagerly at the
        end of the attention phase so the psum accumulator frees early."""
        rows0 = qb * QBLK
        nc_cols = c1 - c0
        cols = slice(c0, c1)
        state = {}
        ops = []
        ln1 = layer_norm_T_ops(xz[:, 0, cols], xz[:, 1, cols], g1_t, be1_t, x[:, cols])
        ops.extend(ln1)

        def ffn_start():
            state["ffn"] = acc_ps.tile([128, nc_cols], f32, tag="acc", name="ps_ffn")

        ops.append(ffn_start)
        for fb in range(FBLK):
            def ffn_chunk(fb=fb):
                ps_h = misc_ps.tile([128, nc_cols], f32, tag="misc", name="ps_h")
                nc.tensor.matmul(
                    ps_h, w1_sb[:, fb * 128 : (fb + 1) * 128], x[:, cols]
                )
                h_sb = h_pool.tile([128, nc_cols], MMDT, tag="h", name="h_sb")
                # relu(x + b1): fused add+max on DVE keeps ACT free for exp
                nc.vector.tensor_scalar(
                    h_sb, ps_h, b1_sb[:, fb : fb + 1], 0.0, ALU.add, ALU.max
                )
                nc.tensor.matmul(
                    state["ffn"],
                    w2_sb[:, fb, :],
                    h_sb,
                    start=(fb == 0),
                    stop=(fb == FBLK - 1),
                    skip_group_check=True,
                )

            ops.append(ffn_chunk)

        def resid():
            state["zz"] = zz = xz_pool.tile([128, 2, nc_cols], MMDT, tag="xz", name="zz")
            nc.vector.tensor_tensor(zz[:, 0, :], state["ffn"], x[:, cols], ALU.add)
            nc.vector.tensor_scalar_add(zz[:, 0, :], zz[:, 0, :], b2_t)
            nc.vector.tensor_tensor(zz[:, 1, :], zz[:, 0, :], zz[:, 0, :], ALU.mult)
            state["y"] = y_pool.tile([128, nc_cols], f32, tag="y", name="y")

        ops.append(resid)

        def ln2_first():
            state["ln2"] = layer_norm_T_ops(
                state["zz"][:, 0, :], state["zz"][:, 1, :], g2_t, be2_t, state["y"]
            )
            state["ln2"][0]()

        ops.append(ln2_first)
        for i in range(1, 6):
            ops.append(lambda i=i: state["ln2"][i]())

        for t in range(nc_cols // 128):
            def store_tile(t=t):
                ps_o = misc_ps.tile([128, 128], f32, tag="misc", name="ps_o")
                nc.tensor.transpose(ps_o, state["y"][:, t * 128 : (t + 1) * 128], ident)
                o_sb = o_pool.tile([128, 128], f32, tag="o", name="o_sb")
                nc.vector.tensor_copy(o_sb, ps_o)
                r0 = rows0 + c0 + t * 128
                nc.sync.dma_start(out=out[r0 : r0 + 128, :], in_=o_sb)

            ops.append(store_tile)
        return ops

    # ---------------- software-pipelined main loop ----------------
    # Per-slot extras: block 0 weaves in the k transposes / v casts it needs
    # (chunk-paced behind the DMAs); later blocks weave in the previous block's
    # post ops (front-loaded over the first ~half of the slots) and the next
    # block's q-column transposes.
    pending = deque()  # post ops of the previous block
    n_slots = NKT // 2
    for qb in range(NQB):
        rows = slice(qb * QBLK, (qb + 1) * QBLK)
        ps_attn = acc_ps.tile([128, QBLK], f32, tag="acc")
        per_slot = 2  # even spread; leftovers carry across the block boundary
        prev_p = None
        for jp in range(n_slots):
            if qb == 0:
                transpose_tile(kT, k_stage, 2 * jp)
                transpose_tile(kT, k_stage, 2 * jp + 1)
                if jp % 4 == 0:  # v cast for the 8 kpos tiles covering this chunk
                    c = slice(2 * jp, 2 * jp + KCH)
                    nc.vector.tensor_copy(v_sb[:, c, :], v_f[:, c, :])
                if jp == 2:
                    nc.sync.dma_start(out=q_stage[:, 8:NQT, :], in_=q_r[:, 8:NQT, :])
                if 3 <= jp < 7:  # block 1's q columns
                    transpose_tile(qT, q_stage, jp + 1)
            ps_s = score_ps.tile([128, 2, QBLK], f32, tag="score")
            for hh in range(2):
                jk = 2 * jp + hh
                nc.tensor.matmul(
                    ps_s[:, hh, :], kT[:, jk * 128 : (jk + 1) * 128], qT[:, rows]
                )
            p_sb = p_pool.tile([128, 2, QBLK], ATTN_DT, tag="p")
            nc.scalar.activation(p_sb, ps_s, AF.Exp, scale=INV_SQRT_D)
            # One-slot skew: accumulate the PREVIOUS pair's P@v so the PE never
            # waits on this slot's exp.
            if prev_p is not None:
                for hh in range(2):
                    jk = 2 * (jp - 1) + hh
                    nc.tensor.matmul(
                        ps_attn,
                        v_sb[:, jk, :],
                        prev_p[:, hh, :],
                        start=(jk == 0),
                        stop=False,
                        skip_group_check=True,
                    )
            prev_p = p_sb
            if jp >= 1:
                for _ in range(per_slot):
                    if pending:
                        pending.popleft()()
        for hh in range(2):  # drain the skewed last pair
            jk = 2 * (n_slots - 1) + hh
            nc.tensor.matmul(
                ps_attn,
                v_sb[:, jk, :],
                prev_p[:, hh, :],
                start=False,
                stop=(hh == 1),
                skip_group_check=True,
            )
        # Eagerly spill the attention accumulator so its psum bank frees for the
        # next block, and square it for the LN1 stats. Remaining post ops carry
        # over into the next block's slots instead of clumping at the boundary.
        xz = xz_pool.tile([128, 2, QBLK], MMDT, tag="xz", name="xz")
        nc.vector.tensor_copy(xz[:, 0, :], ps_attn)
        nc.vector.tensor_tensor(xz[:, 1, :], xz[:, 0, :], xz[:, 0, :], ALU.mult)
        x = x_pool.tile([128, QBLK], MMDT, tag="x", name="x")
        if qb == 0:
            for t in range(8, 12):
                pending.append(lambda t=t: transpose_tile(qT, q_stage, t))
        elif qb == 1:
            for t in range(12, NQT):
                pending.append(lambda t=t: transpose_tile(qT, q_stage, t))
        if qb < NQB - 1:
            pending.extend(make_post_ops(qb, xz, x, 0, QBLK))
        else:
            # split the final block's post phase into two half-width chains so the
            # kernel tail pipelines instead of one long dependency chain
            opsA = make_post_ops(qb, xz, x, 0, QBLK // 2)
            opsB = make_post_ops(qb, xz, x, QBLK // 2, QBLK)
            for a, b in zip(opsA, opsB):
                pending.append(a)
                pending.append(b)
    while pending:
        pending.popleft()()


def _patched_act_tables(module_arch):
    """Collapse the ACT table choice to the one set containing exp+ln (+relu/copy
    fillers) so the kernel never swaps table sets (~2.7us per swap). Positions are
    preserved because act_func_set_id indexes the original act_info.json order."""
    from concourse.hw_specs import get_activation_tables

    tables = get_activation_tables(module_arch)
    keep = "natural_log_exp_and_others"
    if keep in tables:
        return {
            name: (funcs if name == keep else set())
            for name, funcs in tables.items()
        }
    return tables


def build():
    nc = bacc.Bacc("TRN2", target_bir_lowering=False, debug=False, num_devices=N_CORES)
    with tile.TileContext(nc) as tc:
        with ExitStack() as ctx:
            _emit(nc, tc, ctx)
    import concourse.bacc as bacc_mod

    orig = bacc_mod.get_activation_tables
    bacc_mod.get_activation_tables = _patched_act_tables
    try:
        nc.compile()
    finally:
        bacc_mod.get_activation_tables = orig
    return nc


_CACHE = {}


def _get_nc():
    if "nc" not in _CACHE:
        _CACHE["nc"] = build()
    return _CACHE["nc"]


def run(inputs, trace=False, trace_kwargs=None):
    """Run on 8 cores; returns (full_output, BassKernelResults)."""
    nc = _get_nc()
    q = np.asarray(inputs["q"], dtype=np.float32)
    k = np.asarray(inputs["k"], dtype=np.float32)
    v = np.asarray(inputs["v"], dtype=np.float32)
    flat = {
        name: np.ascontiguousarray(np.asarray(inputs[name], dtype=np.float32))
        for name in ("w1", "b1", "w2", "b2", "g1", "be1", "g2", "be2")
    }
    in_maps = []
    for c in range(N_CORES):
        b, h = divmod(c, 2)
        m = dict(flat)
        m["q"] = np.ascontiguousarray(q[b, h * HALF : (h + 1) * HALF, :])
        m["k"] = np.ascontiguousarray(k[b])
        m["v"] = np.ascontiguousarray(v[b])
        in_maps.append(m)
    res = run_bass_kernel_spmd(
        nc, in_maps, list(range(N_CORES)), trace=trace, **(trace_kwargs or {})
    )
    full = np.empty((B, S, D), dtype=np.float32)
    for c in range(N_CORES):
        b, h = divmod(c, 2)
        full[b, h * HALF : (h + 1) * HALF, :] = res.results[c]["out"]
    return full, res


def kernel(**inputs):
    full, _ = run(inputs, trace=False)
    return full

